# revision 1
# baseline (speedup 1.0000x reference)
"""Causal self-attention (B=4, T=2048, D=1024, H=16, rope) on 8 trn2 cores.

Sharding: DP over batch (4) x TP over heads (2 groups of 8). Core c handles
batch c//2, heads [8*(c%2), 8*(c%2)+8). Host pre-slices/transposes weights,
sums the two partial out-projections per batch afterwards.

Per-core kernel (v2):
  - q/k projected d-major [feat, tok] from xT/wqkT; RoPE rotate-half applied
    with partition-offset DVE muls (no swap copies); v projected token-major
    [tok, 65*8] with a ones column per head so softmax denominators ride the
    attention matmul for free.
  - scores transposed (keys on partitions): S_T = k_blk.T @ q per 128-key
    block, two blocks share one 2-bank PSUM tile; one ACT Exp per pair
    (scale=1/8 folded). Causal: fully-masked blocks skipped, diagonal pairs
    column-trimmed, remaining mask applied multiplicatively post-exp on DVE.
  - O_T[d, q] accumulates over key blocks in PSUM; row 64 = softmax sums;
    divide via ones-matmul partition broadcast + DVE reciprocal/mul.
  - out-projection per 512-token block -> yT [1024, 2048] partial.

All matmuls float32r (11-bit rne mantissa, 4x the fp32 matmul rate); inputs
enter f32r tiles via bitcast DMA (PE rounds on ingest).
"""
import sys

for _p in ("/opt/trn_rl_repo", "/root/.axon_site/_ro/trn_rl_repo"):
    if _p not in sys.path:
        sys.path.insert(0, _p)

import numpy as np
import concourse.bass as bass
import concourse.mybir as mybir
import concourse.tile as tile
from concourse.bass_utils import run_bass_kernel_spmd

D_MODEL = 1024
N_HEADS = 16
HEAD_DIM = 64
T = 2048
B = 4
N_CORES = 8
HPC = 8            # heads per core
VW = HEAD_DIM + 1  # v width per head incl ones column
MM_DT = mybir.dt.float32r
F32 = mybir.dt.float32

_CACHED = {}


def split_multi_waits(nc):
    """walrus in this container encodes at most ONE sync wait per
    instruction; hoist extras onto same-engine NOPs placed just before."""
    fn = nc.m.functions[0]
    for bb in fn.blocks:
        new_list = []
        changed = False
        for inst in list(bb.instructions):
            si = getattr(inst, "sync_info", None)
            waits = list(si.on_wait) if (si and si.on_wait) else []
            if len(waits) > 1:
                changed = True
                for w in waits[:-1]:
                    nop = mybir.InstNoOp(
                        name=f"waitnop-{nc.next_id()}", ins=[], outs=[]
                    )
                    nop.engine = inst.engine
                    nop.sync_info = mybir.SyncInfo(on_wait=[w], on_update=[])
                    nc.register_instruction(nop)
                    new_list.append(nop)
                si.on_wait = [waits[-1]]
            new_list.append(inst)
        if changed:
            bb.instructions = new_list


def build_nc():
    nc = bass.Bass()
    xT = nc.declare_dram_parameter("xT", [D_MODEL, T], F32, isOutput=False)
    wqkT = nc.declare_dram_parameter("wqkT", [D_MODEL, 1024], F32, isOutput=False)
    wvT = nc.declare_dram_parameter("wvT", [D_MODEL, HPC * VW], F32, isOutput=False)
    woT = nc.declare_dram_parameter("woT", [HPC * HEAD_DIM, D_MODEL], F32, isOutput=False)
    cosT = nc.declare_dram_parameter("cosT", [128, T], F32, isOutput=False)
    sinT = nc.declare_dram_parameter("sinT", [128, T], F32, isOutput=False)
    masks = nc.declare_dram_parameter("masks", [4, 2, 128, 512], F32, isOutput=False)
    ones_d = nc.declare_dram_parameter("ones_d", [128, 128], F32, isOutput=False)
    yT = nc.declare_dram_parameter("yT", [D_MODEL, T], F32, isOutput=True)

    r = lambda ap: ap.bitcast(MM_DT)
    Exp = mybir.ActivationFunctionType.Exp
    xT_k = xT.rearrange("(kb p) t -> p kb t", p=128)      # [128, 8, T]
    wqkT_k = wqkT.rearrange("(kb p) f -> p kb f", p=128)  # [128, 8, 1024]
    wvT_k = wvT.rearrange("(kb p) f -> p kb f", p=128)    # [128, 8, 520]
    woT_k = woT.rearrange("(fb p) o -> p fb o", p=128)    # [128, 4, 1024]
    masks_k = masks.rearrange("j c p t -> p j c t")       # [128, 4, 2, 512]
    yT_k = yT.rearrange("(ob p) t -> p ob t", p=128)      # [128, 8, T]

    with tile.TileContext(nc) as tc:
        with tc.tile_pool(name="persist", bufs=1) as pp:
            qk_tiles = [pp.tile([128, T], MM_DT, tag=f"qk{m}", name=f"qk{m}")
                        for m in range(8)]
            v_all = pp.tile([128, 16, HPC * VW], MM_DT, tag="v_all", name="v_all")
            ones64 = pp.tile([1, 64], MM_DT, tag="ones64", name="ones64")
            nc.sync.dma_start(out=ones64, in_=r(ones_d[0:1, 0:64]))

            # ---------------- projection phase ----------------
            with tc.tile_pool(name="wpool", bufs=1) as wp, \
                 tc.tile_pool(name="xpool", bufs=2) as xp, \
                 tc.tile_pool(name="cspool", bufs=2) as csp, \
                 tc.tile_pool(name="ptmp", bufs=2) as ptmp, \
                 tc.tile_pool(name="pps", bufs=2, space="PSUM") as pps, \
                 tc.tile_pool(name="ppsv", bufs=2, space="PSUM") as ppsv:
                wqk_all = wp.tile([128, 8, 1024], MM_DT, tag="wqk", name="wqk_all")
                wv_all = wp.tile([128, 8, HPC * VW], MM_DT, tag="wv", name="wv_all")
                for kb in range(8):
                    nc.sync.dma_start(
                        out=wqk_all[:, kb, :], in_=r(wqkT_k[:, kb, :])
                    )
                nc.sync.dma_start(out=wv_all, in_=r(wvT_k))

                for p in range(4):  # token phases of 512
                    ts0 = 512 * p
                    tsl = slice(ts0, ts0 + 512)
                    xt = xp.tile([128, 8, 512], MM_DT, tag="xt", name="xt")
                    for kb in range(8):
                        nc.sync.dma_start(
                            out=xt[:, kb, :], in_=r(xT_k[:, kb, tsl])
                        )
                    cos_sb = csp.tile([128, 512], F32, tag="cos")
                    sin_sb = csp.tile([128, 512], F32, tag="sin")
                    nc.sync.dma_start(out=cos_sb, in_=cosT[:, tsl])
                    nc.sync.dma_start(out=sin_sb, in_=sinT[:, tsl])

                    # q/k d-major + rope
                    for m in range(8):
                        ps = pps.tile([128, 512], F32, tag="psqk")
                        for kb in range(8):
                            nc.tensor.matmul(
                                ps,
                                wqk_all[:, kb, 128 * m:128 * m + 128],
                                xt[:, kb, :],
                                start=(kb == 0),
                                stop=(kb == 7),
                            )
                        raw = ptmp.tile([128, 512], F32, tag="raw")
                        nc.scalar.copy(raw, ps)
                        # rotate-half partner via partition-offset reads:
                        # rows 0:32 <- raw[32:64], 32:64 <- raw[0:32], etc.
                        tb = ptmp.tile([128, 512], F32, tag="tb")
                        for c in range(4):
                            o0 = 32 * c
                            i0 = 32 * (c + 1) if c % 2 == 0 else 32 * (c - 1)
                            nc.gpsimd.tensor_mul(
                                tb[o0:o0 + 32, :], raw[i0:i0 + 32, :],
                                sin_sb[i0:i0 + 32, :],
                            )
                        nc.vector.tensor_mul(raw, raw, cos_sb)
                        nc.vector.tensor_add(qk_tiles[m][:, tsl], raw, tb)

                    # v token-major (4 token tiles of 128 in this phase)
                    for vt in range(4):
                        vg = 4 * p + vt
                        vsl = slice(128 * vt, 128 * vt + 128)
                        psv = ppsv.tile([128, 2, 260], F32, tag="psv",
                                        padded_shape=[128, 2, 512])
                        for c in range(2):
                            for kb in range(8):
                                nc.tensor.matmul(
                                    psv[:, c, :],
                                    xt[:, kb, vsl],
                                    wv_all[:, kb, 260 * c:260 * c + 260],
                                    start=(kb == 0),
                                    stop=(kb == 7),
                                )
                        nc.vector.tensor_copy(v_all[:, vg, :], psv)
                # ones columns for all 16 v tiles in one DMA
                ones_ap = v_all.rearrange(
                    "p vg (h d) -> p vg h d", d=VW
                )[:, :, :, HEAD_DIM]
                nc.sync.dma_start(
                    out=ones_ap,
                    in_=r(ones_d.rearrange("p (vg h) -> p vg h", h=HPC)),
                )

            # ---------------- attention + out-proj ----------------
            with tc.tile_pool(name="wopool", bufs=1) as wop, \
                 tc.tile_pool(name="apool", bufs=2) as apool, \
                 tc.tile_pool(name="epool", bufs=4) as ep, \
                 tc.tile_pool(name="dtmp", bufs=3) as dtmp, \
                 tc.tile_pool(name="ypool", bufs=2) as yp, \
                 tc.tile_pool(name="sps", bufs=2, space="PSUM") as sps, \
                 tc.tile_pool(name="ops", bufs=1, space="PSUM") as ops, \
                 tc.tile_pool(name="yps", bufs=1, space="PSUM") as yps, \
                 tc.tile_pool(name="bcps", bufs=1, space="PSUM") as bcps:
                wo_all = wop.tile([128, 4, 1024], MM_DT, tag="wo", name="wo_all")
                nc.sync.dma_start(out=wo_all, in_=r(woT_k))
                masks_sb = wop.tile([128, 4, 2, 512], F32, tag="masks", name="masks_sb")
                nc.sync.dma_start(out=masks_sb, in_=masks_k)

                for qt in (3, 2, 1, 0):
                    qs0 = 512 * qt
                    aT = apool.tile([128, 4, 512], MM_DT, tag="aT", name="aT")
                    # head pairs: even head at partitions 0:64, odd at 64:128.
                    # Adjacent S-matmuls land in disjoint PE row-groups and
                    # run concurrently (tile_position auto-derived from
                    # base_partition).
                    for hp in range(4):
                        qtile = qk_tiles[hp]
                        ktile = qk_tiles[4 + hp]
                        n_kb = 4 * (qt + 1)
                        o_ps = [
                            ops.tile([65, 512], F32, tag=f"o{i}", name=f"o{i}")
                            for i in range(2)
                        ]
                        for kb in range(n_kb):
                            j = kb - 4 * qt  # >=0 on the diagonal
                            col0 = (0, 128, 256, 256)[j] if j >= 0 else 0
                            csl = slice(col0, 512)
                            s2 = sps.tile([128, 2, 512], F32, tag="s")
                            for i in range(2):
                                rsl = slice(64 * i, 64 * i + 64)
                                nc.tensor.matmul(
                                    s2[:, i, csl],
                                    ktile[rsl, 128 * kb:128 * kb + 128],
                                    qtile[rsl, qs0 + col0:qs0 + 512],
                                    start=True,
                                    stop=True,
                                )
                            expS = ep.tile([128, 2, 512], MM_DT, tag="e")
                            nc.scalar.activation(
                                expS[:, :, csl], s2[:, :, csl], Exp, scale=0.125
                            )
                            if j >= 0:
                                nc.vector.tensor_mul(
                                    expS[:, :, csl],
                                    expS[:, :, csl],
                                    masks_sb[:, j, :, csl],
                                )
                            for i in range(2):
                                h = 2 * hp + i
                                nc.tensor.matmul(
                                    o_ps[i][:, csl],
                                    v_all[:, kb, VW * h:VW * h + VW],
                                    expS[:, i, csl],
                                    start=(kb == 0),
                                    stop=(kb == n_kb - 1),
                                )
                        for i in range(2):
                            # one copy releases the PSUM accumulator early;
                            # divide chain runs off the SBUF copy
                            o_sb = dtmp.tile([64, 512], MM_DT, tag="osb")
                            nc.scalar.copy(o_sb, o_ps[i][0:64, :])
                            sums = dtmp.tile([1, 512], F32, tag="sums")
                            nc.vector.tensor_copy(sums, o_ps[i][64:65, :])
                            rsum = dtmp.tile([1, 512], MM_DT, tag="rsum")
                            with nc.allow_low_precision("f32r is full-width"):
                                nc.vector.reciprocal(rsum, sums)
                            bc = bcps.tile([64, 512], F32, tag="bc")
                            nc.tensor.matmul(bc, ones64, rsum, start=True, stop=True)
                            nc.vector.tensor_mul(
                                aT[64 * i:64 * i + 64, hp, :],
                                o_sb,
                                bc,
                            )
                    for g in range(2):
                        y_all = yp.tile([128, 4, 512], F32, tag="y_all", name="y_all")
                        for ob4 in range(4):
                            ob = 4 * g + ob4
                            y_ps = yps.tile([128, 512], F32, tag="y")
                            for fb in range(4):
                                nc.tensor.matmul(
                                    y_ps,
                                    wo_all[:, fb, 128 * ob:128 * ob + 128],
                                    aT[:, fb, :],
                                    start=(fb == 0),
                                    stop=(fb == 3),
                                )
                            nc.scalar.copy(y_all[:, ob4, :], y_ps)
                        nc.sync.dma_start(
                            out=yT_k[:, 4 * g:4 * g + 4, qs0:qs0 + 512], in_=y_all
                        )

    split_multi_waits(nc)
    nc.finalize()
    return nc


def host_inputs(x, w_qkv, w_out):
    """Per-core input dicts."""
    x = np.asarray(x, dtype=np.float32)
    w_qkv = np.asarray(w_qkv, dtype=np.float32)
    w_out = np.asarray(w_out, dtype=np.float32)

    theta = 1.0 / (10000.0 ** (np.arange(0, HEAD_DIM, 2, dtype=np.float32) / HEAD_DIM))
    t = np.arange(T, dtype=np.float32)
    freqs = np.outer(t, theta)  # [T, 32]
    cos32 = np.cos(freqs).astype(np.float32).T  # [32, T]
    sin32 = np.sin(freqs).astype(np.float32).T
    cosT = np.tile(cos32, (4, 1))  # [128, T] rows r -> freq r%32
    sinT = np.empty((128, T), dtype=np.float32)
    for blk in range(4):
        sgn = -1.0 if blk % 2 == 0 else 1.0
        sinT[32 * blk:32 * blk + 32] = sgn * sin32
    # kernel reads sin at the rotate-half SOURCE partition offset (gpsimd
    # needs equal input base partitions) -> swap 32-row blocks pairwise
    sinT = sinT[[*range(32, 64), *range(0, 32), *range(96, 128), *range(64, 96)], :]

    # multiplicative masks: plane j allows q_l >= k_l + 128*j,
    # duplicated along a 2-wide head lane for paired-head tiles
    masks1 = np.zeros((4, 128, 512), dtype=np.float32)
    kl = np.arange(128)[:, None]
    ql = np.arange(512)[None, :]
    for j in range(4):
        masks1[j] = (ql >= kl + 128 * j).astype(np.float32)
    masks = np.ascontiguousarray(np.repeat(masks1[:, None], 2, axis=1))

    maps = []
    for c in range(N_CORES):
        b, g = divmod(c, 2)
        heads = range(HPC * g, HPC * g + HPC)
        q_rows = np.concatenate([np.arange(64 * h, 64 * h + 64) for h in heads])
        wqkT = np.concatenate(
            [w_qkv[q_rows, :], w_qkv[1024 + q_rows, :]], axis=0
        ).T.copy()  # [1024, 1024]
        wv = w_qkv[2048 + q_rows, :]  # [512, 1024]
        wvT = np.zeros((D_MODEL, HPC * VW), dtype=np.float32)
        for lh in range(HPC):
            wvT[:, VW * lh:VW * lh + HEAD_DIM] = wv[64 * lh:64 * lh + 64, :].T
        woT = w_out[:, q_rows].T.copy()  # [512, 1024]
        maps.append({
            "xT": np.ascontiguousarray(x[b].T),
            "wqkT": np.ascontiguousarray(wqkT),
            "wvT": wvT,
            "woT": np.ascontiguousarray(woT),
            "cosT": np.ascontiguousarray(cosT),
            "sinT": np.ascontiguousarray(sinT),
            "masks": masks,
            "ones_d": np.ones((128, 128), dtype=np.float32),
        })
    return maps


def assemble(results):
    y = np.empty((B, T, D_MODEL), dtype=np.float32)
    for b in range(B):
        yT = results[2 * b]["yT"] + results[2 * b + 1]["yT"]
        y[b] = yT.T
    return y


def kernel(x, w_qkv, w_out):
    if "nc" not in _CACHED:
        _CACHED["nc"] = build_nc()
    nc = _CACHED["nc"]
    maps = host_inputs(x, w_qkv, w_out)
    res = run_bass_kernel_spmd(nc, maps, list(range(N_CORES)))
    return assemble(res.results)



# revision 7
# speedup vs baseline: 10.6432x; 10.6432x over previous
"""Causal self-attention (B=4, T=2048, D=1024, H=16, rope) on 8 trn2 cores.

Sharding: DP over batch (4) x TP over heads (2 groups of 8). Core c handles
batch c//2, heads [8*(c%2), 8*(c%2)+8). Host pre-slices/transposes/bf16-casts
weights, sums the two partial out-projections per batch afterwards.

Per-core kernel (v3, bf16):
  - q/k projected d-major [feat, tok] from bf16 xT/wqkT; RoPE rotate-half via
    partition-offset gpsimd muls + DVE mul/add, all bf16; v projected
    token-major [tok, 65*8] with a ones column per head (softmax denominators
    ride the AV matmul).
  - scores transposed (keys on partitions): S_T = k_blk.T @ q per 128-key
    block, head pairs in disjoint PE row groups; one ACT Exp (scale=1/8) to
    bf16. Causal: fully-masked blocks skipped, diagonal blocks column-trimmed
    and triangle-masked by one canonical [128,2,128] DVE multiply.
  - AV in O-layout: O[q,f] accumulates per 128-q chunk with expS as the
    stationary operand and v token-major as moving (65-row matmuls); col 64 =
    denominators. Divide = DVE per-partition reciprocal+scale (token-major),
    then transpose to d-major via identity matmul; gpsimd assembles aT.
  - out-projection per 512-token block -> yT bf16 partial [1024, 2048].
"""
import sys

for _p in ("/opt/trn_rl_repo", "/root/.axon_site/_ro/trn_rl_repo"):
    if _p not in sys.path:
        sys.path.insert(0, _p)

import contextlib

import ml_dtypes
import numpy as np

import concourse.bass as bass
import concourse.mybir as mybir
import concourse.tile as tile
from concourse.bass_utils import run_bass_kernel_spmd

D_MODEL = 1024
N_HEADS = 16
HEAD_DIM = 64
T = 2048
B = 4
N_CORES = 8
HPC = 8            # heads per core
VW = HEAD_DIM + 1  # v width per head incl ones column
BF16 = mybir.dt.bfloat16
F32 = mybir.dt.float32
NPBF = ml_dtypes.bfloat16

_CACHED = {}


def split_multi_waits(nc):
    """walrus in this container encodes at most ONE sync wait per
    instruction; hoist extras onto same-engine NOPs placed just before."""
    fn = nc.m.functions[0]
    for bb in fn.blocks:
        new_list = []
        changed = False
        for inst in list(bb.instructions):
            si = getattr(inst, "sync_info", None)
            waits = list(si.on_wait) if (si and si.on_wait) else []
            if len(waits) > 1:
                changed = True
                for w in waits[:-1]:
                    nop = mybir.InstNoOp(
                        name=f"waitnop-{nc.next_id()}", ins=[], outs=[]
                    )
                    nop.engine = inst.engine
                    nop.sync_info = mybir.SyncInfo(on_wait=[w], on_update=[])
                    nc.register_instruction(nop)
                    new_list.append(nop)
                si.on_wait = [waits[-1]]
            new_list.append(inst)
        if changed:
            bb.instructions = new_list


def build_nc(loop_reps=None):
    nc = bass.Bass()
    xT = nc.declare_dram_parameter("xT", [D_MODEL, T], BF16, isOutput=False)
    wqkT = nc.declare_dram_parameter("wqkT", [D_MODEL, 1024], BF16, isOutput=False)
    wvT = nc.declare_dram_parameter("wvT", [D_MODEL, HPC * VW], BF16, isOutput=False)
    woT = nc.declare_dram_parameter("woT", [HPC * HEAD_DIM, D_MODEL], BF16, isOutput=False)
    cosT = nc.declare_dram_parameter("cosT", [128, T], BF16, isOutput=False)
    sinT = nc.declare_dram_parameter("sinT", [128, T], BF16, isOutput=False)
    masks2 = nc.declare_dram_parameter("masks2", [128, 2, 128], BF16, isOutput=False)
    ident_d = nc.declare_dram_parameter("ident_d", [128, 128], BF16, isOutput=False)
    ones_d = nc.declare_dram_parameter("ones_d", [128, 128], BF16, isOutput=False)
    yT = nc.declare_dram_parameter("yT", [D_MODEL, T], BF16, isOutput=True)

    Exp = mybir.ActivationFunctionType.Exp
    xT_k = xT.rearrange("(kb p) t -> p kb t", p=128)      # [128, 8, T]
    wqkT_k = wqkT.rearrange("(kb p) f -> p kb f", p=128)  # [128, 8, 1024]
    wvT_k = wvT.rearrange("(kb p) f -> p kb f", p=128)    # [128, 8, 520]
    woT_k = woT.rearrange("(fb p) o -> p fb o", p=128)    # [128, 4, 1024]
    yT_k = yT.rearrange("(ob p) t -> p ob t", p=128)      # [128, 8, T]

    with tile.TileContext(nc) as tc:
      with (tc.For_i(0, loop_reps) if loop_reps else contextlib.nullcontext()):
        with tc.tile_pool(name="persist", bufs=1) as pp:
            qk_tiles = [pp.tile([128, T], BF16, tag=f"qk{m}", name=f"qk{m}")
                        for m in range(8)]
            v_all = pp.tile([128, 16, HPC * VW], BF16, tag="v_all", name="v_all")
            ident = pp.tile([128, 128], BF16, tag="ident", name="ident")
            masks_sb = pp.tile([128, 2, 128], BF16, tag="masks", name="masks_sb")
            nc.sync.dma_start(out=ident, in_=ident_d[:, :])
            nc.sync.dma_start(out=masks_sb, in_=masks2[:, :, :])

            # ---------------- projection phase ----------------
            with tc.tile_pool(name="wpool", bufs=1) as wp, \
                 tc.tile_pool(name="xpool", bufs=2) as xp, \
                 tc.tile_pool(name="cspool", bufs=2) as csp, \
                 tc.tile_pool(name="ptmp", bufs=2) as ptmp, \
                 tc.tile_pool(name="pps", bufs=2, space="PSUM") as pps, \
                 tc.tile_pool(name="ppsv", bufs=2, space="PSUM") as ppsv:
                wqk_all = wp.tile([128, 8, 1024], BF16, tag="wqk", name="wqk_all")
                wv_all = wp.tile([128, 8, HPC * VW], BF16, tag="wv", name="wv_all")
                for kb in range(8):
                    nc.sync.dma_start(
                        out=wqk_all[:, kb, :], in_=wqkT_k[:, kb, :]
                    )
                nc.sync.dma_start(out=wv_all, in_=wvT_k)

                for p in range(4):  # token phases of 512
                    ts0 = 512 * p
                    tsl = slice(ts0, ts0 + 512)
                    xt = xp.tile([128, 8, 512], BF16, tag="xt", name="xt")
                    for kb in range(8):
                        nc.sync.dma_start(
                            out=xt[:, kb, :], in_=xT_k[:, kb, tsl]
                        )
                    cos_sb = csp.tile([128, 512], BF16, tag="cos")
                    sin_sb = csp.tile([128, 512], BF16, tag="sin")
                    nc.sync.dma_start(out=cos_sb, in_=cosT[:, tsl])
                    nc.sync.dma_start(out=sin_sb, in_=sinT[:, tsl])

                    # q/k d-major + rope
                    for m in range(8):
                        ps = pps.tile([128, 512], F32, tag="psqk")
                        for kb in range(8):
                            nc.tensor.matmul(
                                ps,
                                wqk_all[:, kb, 128 * m:128 * m + 128],
                                xt[:, kb, :],
                                start=(kb == 0),
                                stop=(kb == 7),
                            )
                        raw = ptmp.tile([128, 512], BF16, tag="raw")
                        nc.scalar.copy(raw, ps)
                        # rotate-half partner via partition-offset reads:
                        # rows 0:32 <- raw[32:64], 32:64 <- raw[0:32], etc.
                        tb = ptmp.tile([128, 512], BF16, tag="tb")
                        for c in range(4):
                            o0 = 32 * c
                            i0 = 32 * (c + 1) if c % 2 == 0 else 32 * (c - 1)
                            nc.gpsimd.tensor_mul(
                                tb[o0:o0 + 32, :], raw[i0:i0 + 32, :],
                                sin_sb[i0:i0 + 32, :],
                            )
                        nc.vector.tensor_mul(raw, raw, cos_sb)
                        nc.vector.tensor_add(qk_tiles[m][:, tsl], raw, tb)

                    # v token-major (4 token tiles of 128 in this phase)
                    for vt in range(4):
                        vg = 4 * p + vt
                        vsl = slice(128 * vt, 128 * vt + 128)
                        psv = ppsv.tile([128, 2, 260], F32, tag="psv",
                                        padded_shape=[128, 2, 512])
                        for c in range(2):
                            for kb in range(8):
                                nc.tensor.matmul(
                                    psv[:, c, :],
                                    xt[:, kb, vsl],
                                    wv_all[:, kb, 260 * c:260 * c + 260],
                                    start=(kb == 0),
                                    stop=(kb == 7),
                                )
                        nc.vector.tensor_copy(v_all[:, vg, :], psv)
                # ones columns for all 16 v tiles in one DMA
                ones_ap = v_all.rearrange(
                    "p vg (h d) -> p vg h d", d=VW
                )[:, :, :, HEAD_DIM]
                nc.sync.dma_start(
                    out=ones_ap,
                    in_=ones_d.rearrange("p (vg h) -> p vg h", h=HPC),
                )

            # ---------------- attention + out-proj ----------------
            with tc.tile_pool(name="wopool", bufs=1) as wop, \
                 tc.tile_pool(name="apool", bufs=2) as apool, \
                 tc.tile_pool(name="epool", bufs=4) as ep, \
                 tc.tile_pool(name="dtmp", bufs=3) as dtmp, \
                 tc.tile_pool(name="ypool", bufs=2) as yp, \
                 tc.tile_pool(name="sps", bufs=2, space="PSUM") as sps, \
                 tc.tile_pool(name="ops", bufs=1, space="PSUM") as ops, \
                 tc.tile_pool(name="tps", bufs=1, space="PSUM") as tps, \
                 tc.tile_pool(name="yps", bufs=1, space="PSUM") as yps:
                wo_all = wop.tile([128, 4, 1024], BF16, tag="wo", name="wo_all")
                nc.sync.dma_start(out=wo_all, in_=woT_k)

                for qt in (3, 2, 1, 0):
                    qs0 = 512 * qt
                    n_kb = 4 * (qt + 1)
                    aT = apool.tile([128, 4, 512], BF16, tag="aT", name="aT")
                    for hp in range(4):
                        qtile = qk_tiles[hp]
                        ktile = qk_tiles[4 + hp]
                        o_ps = [
                            ops.tile([128, 4, VW], F32, tag=f"o{i}",
                                     name=f"o{i}", padded_shape=[128, 4, 128])
                            for i in range(2)
                        ]
                        for kb in range(n_kb):
                            j = kb - 4 * qt  # >=0 on the diagonal
                            col0 = (0, 128, 256, 384)[j] if j >= 0 else 0
                            csl = slice(col0, 512)
                            s2 = sps.tile([128, 2, 512], F32, tag="s")
                            for i in range(2):
                                rsl = slice(64 * i, 64 * i + 64)
                                nc.tensor.matmul(
                                    s2[:, i, csl],
                                    ktile[rsl, 128 * kb:128 * kb + 128],
                                    qtile[rsl, qs0 + col0:qs0 + 512],
                                    start=True,
                                    stop=True,
                                )
                            expS = ep.tile([128, 2, 512], BF16, tag="e")
                            nc.scalar.activation(
                                expS[:, :, csl], s2[:, :, csl], Exp, scale=0.125
                            )
                            if j >= 0:
                                tsl = slice(128 * j, 128 * j + 128)
                                nc.vector.tensor_mul(
                                    expS[:, :, tsl], expS[:, :, tsl], masks_sb
                                )
                            for i in range(2):
                                h = 2 * hp + i
                                for qc in range(j if j > 0 else 0, 4):
                                    # one accumulation group per o_ps bank:
                                    # start marks the whole 2KB zero region,
                                    # each qc's first write then overwrites
                                    nc.tensor.matmul(
                                        o_ps[i][:, qc, :],
                                        expS[:, i, 128 * qc:128 * qc + 128],
                                        v_all[:, kb, VW * h:VW * h + VW],
                                        start=(kb == 0 and qc == 0),
                                        stop=(kb == n_kb - 1 and qc == 3),
                                    )
                        for i in range(2):
                            # token-major divide: per-partition reciprocal
                            rec = dtmp.tile([128, 4], F32, tag="rec")
                            with nc.allow_low_precision("denominators >= 1"):
                                nc.vector.reciprocal(
                                    rec, o_ps[i][:, :, HEAD_DIM]
                                )
                            aO = dtmp.tile([128, 4, HEAD_DIM], BF16, tag="aO")
                            for qc in range(4):
                                nc.vector.tensor_scalar_mul(
                                    aO[:, qc, :],
                                    o_ps[i][:, qc, 0:HEAD_DIM],
                                    rec[:, qc:qc + 1],
                                )
                            # transpose to d-major via identity matmul
                            tp = tps.tile([64, 4, 128], F32, tag="tp")
                            for qc in range(4):
                                nc.tensor.matmul(
                                    tp[:, qc, :], aO[:, qc, :], ident,
                                    start=True, stop=True,
                                )
                            nc.vector.tensor_copy(
                                aT[64 * i:64 * i + 64, hp, :], tp
                            )
                    for g in range(2):
                        y_all = yp.tile([128, 4, 512], BF16, tag="y_all", name="y_all")
                        for ob4 in range(4):
                            ob = 4 * g + ob4
                            y_ps = yps.tile([128, 512], F32, tag="y")
                            for fb in range(4):
                                nc.tensor.matmul(
                                    y_ps,
                                    wo_all[:, fb, 128 * ob:128 * ob + 128],
                                    aT[:, fb, :],
                                    start=(fb == 0),
                                    stop=(fb == 3),
                                )
                            nc.scalar.copy(y_all[:, ob4, :], y_ps)
                        nc.sync.dma_start(
                            out=yT_k[:, 4 * g:4 * g + 4, qs0:qs0 + 512], in_=y_all
                        )

    split_multi_waits(nc)
    nc.finalize()
    return nc


def host_inputs(x, w_qkv, w_out):
    """Per-core input dicts (bf16)."""
    x = np.asarray(x, dtype=np.float32)
    w_qkv = np.asarray(w_qkv, dtype=np.float32)
    w_out = np.asarray(w_out, dtype=np.float32)

    theta = 1.0 / (10000.0 ** (np.arange(0, HEAD_DIM, 2, dtype=np.float32) / HEAD_DIM))
    t = np.arange(T, dtype=np.float32)
    freqs = np.outer(t, theta)  # [T, 32]
    cos32 = np.cos(freqs).astype(np.float32).T  # [32, T]
    sin32 = np.sin(freqs).astype(np.float32).T
    cosT = np.tile(cos32, (4, 1))  # [128, T] rows r -> freq r%32
    sinT = np.empty((128, T), dtype=np.float32)
    for blk in range(4):
        sgn = -1.0 if blk % 2 == 0 else 1.0
        sinT[32 * blk:32 * blk + 32] = sgn * sin32
    # kernel reads sin at the rotate-half SOURCE partition offset (gpsimd
    # needs equal input base partitions) -> swap 32-row blocks pairwise
    sinT = sinT[[*range(32, 64), *range(0, 32), *range(96, 128), *range(64, 96)], :]

    # canonical diagonal-block triangle: allow q_local >= k_local,
    # duplicated along a 2-wide head lane
    kl = np.arange(128)[:, None]
    ql = np.arange(128)[None, :]
    tri = (ql >= kl).astype(np.float32)  # [128, 128]
    masks2 = np.ascontiguousarray(
        np.repeat(tri[:, None, :], 2, axis=1)).astype(NPBF)

    maps = []
    for c in range(N_CORES):
        b, g = divmod(c, 2)
        heads = range(HPC * g, HPC * g + HPC)
        q_rows = np.concatenate([np.arange(64 * h, 64 * h + 64) for h in heads])
        wqkT = np.concatenate(
            [w_qkv[q_rows, :], w_qkv[1024 + q_rows, :]], axis=0
        ).T.copy()  # [1024, 1024]
        wv = w_qkv[2048 + q_rows, :]  # [512, 1024]
        wvT = np.zeros((D_MODEL, HPC * VW), dtype=np.float32)
        for lh in range(HPC):
            wvT[:, VW * lh:VW * lh + HEAD_DIM] = wv[64 * lh:64 * lh + 64, :].T
        woT = w_out[:, q_rows].T.copy()  # [512, 1024]
        maps.append({
            "xT": np.ascontiguousarray(x[b].T).astype(NPBF),
            "wqkT": np.ascontiguousarray(wqkT).astype(NPBF),
            "wvT": wvT.astype(NPBF),
            "woT": np.ascontiguousarray(woT).astype(NPBF),
            "cosT": np.ascontiguousarray(cosT).astype(NPBF),
            "sinT": np.ascontiguousarray(sinT).astype(NPBF),
            "masks2": masks2,
            "ident_d": np.eye(128, dtype=np.float32).astype(NPBF),
            "ones_d": np.ones((128, 128), dtype=np.float32).astype(NPBF),
        })
    return maps


def assemble(results):
    y = np.empty((B, T, D_MODEL), dtype=np.float32)
    for b in range(B):
        yT = (results[2 * b]["yT"].astype(np.float32)
              + results[2 * b + 1]["yT"].astype(np.float32))
        y[b] = yT.T
    return y


def kernel(x, w_qkv, w_out):
    if "nc" not in _CACHED:
        _CACHED["nc"] = build_nc()
    nc = _CACHED["nc"]
    maps = host_inputs(x, w_qkv, w_out)
    res = run_bass_kernel_spmd(nc, maps, list(range(N_CORES)))
    return assemble(res.results)
